# revision 32
# baseline (speedup 1.0000x reference)
"""Trainium2 Bass kernel for dynamic per-sample depthwise conv (DPAC).

Reference computation (B=32, C=384, H=W=56, K=7):
  x_avg = mean(x, HW); x_max = max(x, HW)
  x_w   = gelu(x_avg @ w_avg.T + b_avg + x_max @ w_max.T + b_max)
  xr    = x_w[:,:,None] * w_mix  -> [B,C,49]
  Gx    = ||xr||_2 over taps = |x_w| * ||w_mix||_row
  Nx    = Gx / (mean_c(Gx) + eps)
  kern  = gamma * (xr * Nx) + beta  -> [B,C,7,7]
  out   = depthwise_conv(x, kern, pad=3)

Sharding: pure data parallel, batch split across 8 cores, params replicated.
Host-side work is layout only: zero-padding x for the conv halo, a bf16
cast, and pre-transposing/stacking the 1x1-conv weight matrices for the PE.

Phase 3 runs a 4-way engine split balanced against the v2 cost model, all
in bf16 (the rel-err budget is 2e-2; bf16 rounding contributes ~1e-3):
 - 18 taps on PE as bf16 diag-matmuls (1 cy/row)
 - 13 taps on ACT (per-partition mul -> bf16 tmp) accumulated on PE via
   bf16 identity matmuls into the same PSUM chain
 - 9 taps on DVE as fused STT
 - 9 taps on Pool as fused STT into a Pool-local accumulator
x lives in SBUF as bf16 for the whole kernel (loaded once), so phase 3 has
no DMA dependencies; diag stationaries for tile i+1 are built one tile
early so the PE chain never waits on DVE.
"""

import numpy as np
import ml_dtypes
from contextlib import ExitStack

import concourse.bass as bass
import concourse.bass_isa as bass_isa
import concourse.tile as tile
from concourse import mybir
from concourse import bass_utils

B, C, H, W, KW = 32, 384, 56, 56, 7
NCORES = 8
BL = B // NCORES            # samples per core
PAD = KW // 2               # 3
HP, WP = H + 2 * PAD, W + 2 * PAD   # 62, 62
P = 128                     # partitions
CG = C // P                 # channel groups (3)
NTAPS = KW * KW             # 49
EPS = 1e-6

F32 = mybir.dt.float32
BF16 = mybir.dt.bfloat16
AX = mybir.AxisListType
OP = mybir.AluOpType
AF = mybir.ActivationFunctionType


def _prune_redundant_dma_waits(nc, strict=True):
    """Drop DMA sem waits that are transitively implied by another wait.

    The dynamic-DMA descriptor carries only one sem wait (walrus rejects
    more: "Too many sync wait commands"), but Tile's wait assignment is
    per-proc minimal, not transitively minimal (optimize_sems is disabled),
    so recycled-slot DMAs get both a reader-engine wait and the old writer's
    DMA-lane wait even when the former implies the latter.

    Soundness: a wait (P >= t) on an in-order proc P (engine/sequencer)
    guarantees that P's instructions with tick <= t have completed, hence
    their own waits were satisfied; knowledge propagates transitively.
    DMA-lane procs are NOT assumed in-order: a lane wait only contributes
    its own fact.
    """
    import bass_rust as _br
    PROC_NAMES = _br.PROC_NAMES
    name_to_idx = {n: i for i, n in enumerate(PROC_NAMES)}
    dma_procs = {i for i, n in enumerate(PROC_NAMES) if n.startswith("DMA")}
    INC = {i: (16 if i in dma_procs else 1) for i in range(len(PROC_NAMES))}

    def sem_proc(ant_name):
        base = ant_name.rsplit("_", 1)[0]
        return name_to_idx.get(base)

    # per-proc streams in tick order
    streams = {}
    all_insts = []
    for blk in nc.m.functions[0].blocks:
        for ins in blk.instructions:
            all_insts.append(ins)
            p = ins.bass_scheduled_proc
            t = ins.bass_scheduled_tick
            if p is not None and t is not None:
                streams.setdefault(p, []).append((t, ins))
    for p in streams:
        streams[p].sort(key=lambda x: x[0])

    def merge(a, b):
        for k, v in b.items():
            if a.get(k, -1) < v:
                a[k] = v

    # K(ins): facts known when ins starts = from its waits.
    # SK(p, i): cumulative facts after first i insts of in-order proc p.
    K_memo, SK_memo = {}, {}

    def K(ins):
        r = K_memo.get(ins.name)
        if r is not None:
            return r
        K_memo[ins.name] = {}  # cycle guard (schedule is acyclic anyway)
        facts = {}
        si = ins.sync_info
        if si is not None:
            for w in si.on_wait:
                if w.wait_mode != "sem-ge-imm" or w.wait_reg is not None:
                    continue
                p = sem_proc(w.ant_name)
                if p is None:
                    continue
                t = w.wait_value // INC[p]
                merge(facts, {p: t})
                if p not in dma_procs:
                    merge(facts, SK(p, t))
                else:
                    # lane sem >= 16t implies the first t DMAs on the lane
                    # started (per-lane FIFO), hence their waits held
                    merge(facts, DK(p, t))
        K_memo[ins.name] = facts
        return facts

    def SK(p, t):
        # facts after in-order proc p's stream reached tick t
        st = streams.get(p, [])
        i = 0
        while i < len(st) and st[i][0] <= t:
            i += 1
        key = (p, i)
        r = SK_memo.get(key)
        if r is not None:
            return r
        SK_memo[key] = {}  # cycle guard
        if i == 0:
            facts = {}
        else:
            tick_i, ins_i = st[i - 1]
            facts = dict(SK(p, tick_i - 1))
            merge(facts, K(ins_i))
            merge(facts, {p: tick_i})
        SK_memo[key] = facts
        return facts

    def DK(p, t):
        # facts guaranteed when in-order proc p DISPATCHES past tick t:
        # union of K(inst_i) for tick_i <= t (no completion facts; earlier
        # instructions' waits were satisfied at their dispatch, even if
        # they have not completed yet)
        st = streams.get(p, [])
        i = 0
        while i < len(st) and st[i][0] <= t:
            i += 1
        key = ("DK", p, i)
        r = SK_memo.get(key)
        if r is not None:
            return r
        SK_memo[key] = {}
        if i == 0:
            facts = {}
        else:
            tick_i, ins_i = st[i - 1]
            facts = dict(DK(p, tick_i - 1))
            merge(facts, K(ins_i))
        SK_memo[key] = facts
        return facts

    def prune_inst(ins, strict_one):
        si = ins.sync_info
        if si is None or len(si.on_wait) <= 1:
            return 0
        waits = list(si.on_wait)
        if any(w.wait_mode != "sem-ge-imm" or w.wait_reg is not None
               for w in waits):
            if strict_one:
                raise RuntimeError(f"{ins.name}: non-imm wait on {ins.name}")
            return 0
        # implicit knowledge from same-proc program order
        base = {}
        p0, t0 = ins.bass_scheduled_proc, ins.bass_scheduled_tick
        if p0 is not None and t0 is not None and p0 not in dma_procs:
            base = dict(DK(p0, t0 - 1))
            # DVE/ACT/PE execute and complete strictly in order with an
            # output-hazard interlock, so same-engine RAW data is visible
            # without an explicit sem wait
            # Pool (gpsimd) qualifies for partition-aligned elementwise
            # chains: each Q7 core runs its instruction stream in order and
            # only touches its own 16 partitions
            if PROC_NAMES[p0] in ("DVE", "Activation", "PE", "Pool"):
                merge(base, SK(p0, t0 - 1))
                merge(base, {p0: t0 - 1})
        # iteratively drop any wait implied by base + the other waits
        kept = list(waits)
        changed = True
        while changed and len(kept) > 1:
            changed = False
            for w in list(kept):
                others = [o for o in kept if o is not w]
                facts = dict(base)
                for o in others:
                    p = sem_proc(o.ant_name)
                    if p is None:
                        continue
                    t = o.wait_value // INC[p]
                    merge(facts, {p: t})
                    if p not in dma_procs:
                        merge(facts, SK(p, t))
                    else:
                        merge(facts, DK(p, t))
                pw = sem_proc(w.ant_name)
                tw = w.wait_value // INC[pw] if pw is not None else None
                if pw is not None and facts.get(pw, -1) >= tw:
                    kept = others
                    changed = True
                    break
        if strict_one and len(kept) > 1:
            raise RuntimeError(
                f"{ins.name} ({type(ins).__name__}): cannot reduce waits to "
                "one: " + str([(w.ant_name, w.wait_value) for w in kept]))
        if len(kept) < len(waits):
            si.on_wait = kept
            ins.sync_info = si
            return 1
        return 0

    n_pruned = 0
    for ins in all_insts:
        if not ins.is_executable():
            # e.g. Drain: still subject to walrus's wait-count limit, and
            # dropping transitively-implied waits is sound for any
            # instruction
            n_pruned += prune_inst(ins, strict_one=False)
            continue
        n_pruned += prune_inst(
            ins, strict_one=strict and not ins.is_sequencer_only())
    return n_pruned


PE_TAPS = [0, 1, 3, 4, 6, 7, 8, 10, 11, 13, 14, 15, 17, 18, 20, 21, 22, 24, 25, 27, 28, 29, 31, 32, 34, 35, 36, 38, 39]
ACT_TAPS = []
DVE_TAPS = [2, 9, 16, 23, 30, 37, 44, 46, 48, 12, 19, 26, 40, 33, 5, 47, 45, 41, 42, 43]
POOL_TAPS = []
NROW = 8
NCHUNK = H // NROW


def _build_bass():
    nc = bass.Bass("TRN2", target_bir_lowering=False, debug=False,
                   num_devices=NCORES)

    xpb = nc.dram_tensor("xpb", [BL, C, HP, WP], BF16,
                         kind="ExternalInput").ap()
    w2t = nc.dram_tensor("w2t", [2, CG, P, C], F32, kind="ExternalInput").ap()
    b2 = nc.dram_tensor("b2", [2, C], F32, kind="ExternalInput").ap()
    wmx = nc.dram_tensor("wmx", [C, NTAPS], F32, kind="ExternalInput").ap()
    gam = nc.dram_tensor("gam", [C], F32, kind="ExternalInput").ap()
    bet = nc.dram_tensor("bet", [C], F32, kind="ExternalInput").ap()
    ident = nc.dram_tensor("ident", [P, P], BF16, kind="ExternalInput").ap()
    outb = nc.dram_tensor("outb", [BL, C, H, W], BF16,
                          kind="ExternalOutput").ap()

    with tile.TileContext(nc) as tc, ExitStack() as ctx:
        singles = ctx.enter_context(tc.tile_pool(name="singles", bufs=1))
        accp = ctx.enter_context(tc.tile_pool(name="accp", bufs=2))
        tmpp = ctx.enter_context(tc.tile_pool(name="tmpp", bufs=2))
        small = ctx.enter_context(tc.tile_pool(name="small", bufs=14))
        psum = ctx.enter_context(tc.tile_pool(name="psum", bufs=1, space="PSUM"))

        # ---- load params (once) ----
        w2t_sb = singles.tile([P, 2, CG, C], F32)
        nc.sync.dma_start(out=w2t_sb, in_=w2t.rearrange("s g p c -> p s g c"))
        b2_sb = singles.tile([P, 2, CG], F32)
        nc.sync.dma_start(out=b2_sb, in_=b2.rearrange("s (g p) -> p s g", p=P))
        wmx_sb = singles.tile([P, CG, NTAPS], F32)
        nc.sync.dma_start(out=wmx_sb, in_=wmx.rearrange("(g p) t -> p g t", p=P))
        gam_sb = singles.tile([P, CG], F32)
        nc.sync.dma_start(out=gam_sb, in_=gam.rearrange("(g p) -> p g", p=P))
        bet_sb = singles.tile([P, CG], F32)
        nc.sync.dma_start(out=bet_sb, in_=bet.rearrange("(g p) -> p g", p=P))
        ident_bf = singles.tile([P, P], BF16)
        nc.sync.dma_start(out=ident_bf, in_=ident)
        # resident bf16 x: loaded once, used by pooling and the conv
        xres = singles.tile([P, BL, CG, HP, WP], BF16)
        dpool = ctx.enter_context(tc.tile_pool(name="dpool", bufs=64))
        pep = ctx.enter_context(tc.tile_pool(name="pep", bufs=1, space="PSUM"))

        # DVE observers of param loads: thread each load's completion into
        # DVE's knowledge so downstream instruction waits collapse to one sem
        w2t_obs = singles.tile([P, 3], F32)
        nc.vector.tensor_copy(out=w2t_obs[:, 0:1], in_=w2t_sb[:, 0, 0, 0:1])
        id_obs = singles.tile([P, 1], BF16)
        nc.vector.tensor_copy(out=id_obs, in_=ident_bf[:, 0:1])
        nc.vector.tensor_copy(out=w2t_obs[:, 1:2], in_=gam_sb[:, 0:1])
        nc.vector.tensor_copy(out=w2t_obs[:, 2:3], in_=bet_sb[:, 0:1])

        # bias sum b_avg + b_max  [P, CG]
        bsum = singles.tile([P, CG], F32)
        nc.vector.tensor_add(out=bsum, in0=b2_sb[:, 0, :], in1=b2_sb[:, 1, :])

        # row norms of w_mix: wn[c] = sqrt(sum_t w_mix[c,t]^2)  [P, CG]
        wn = singles.tile([P, CG], F32)
        for g in range(CG):
            sq = small.tile([P, NTAPS], F32, tag="sq49")
            nc.vector.tensor_mul(out=sq, in0=wmx_sb[:, g, :], in1=wmx_sb[:, g, :])
            nc.vector.tensor_reduce(out=wn[:, g:g + 1], in_=sq, axis=AX.X, op=OP.add)
        nc.scalar.activation(out=wn, in_=wn, func=AF.Sqrt)

        x2_g = singles.tile([P, 2, CG, BL], F32)
        xw_g = singles.tile([P, CG, BL], F32)
        gx_g = singles.tile([P, CG, BL], F32)
        kern_g = singles.tile([P, CG, BL, NTAPS], F32)
        ones_col = singles.tile([P, 1], F32)
        nc.vector.memset(ones_col, 1.0)
        ones_row = singles.tile([1, P], F32)
        nc.vector.memset(ones_row, 1.0)
        def phase12(blist):
            # ---- phase 1: load x (once) + pooling ----
                # X2[p, s, g, b]: s=0 mean, s=1 max
                x2 = x2_g
                for b in blist:
                    for g in range(CG):
                        xt = xres[:, b, g]
                        nc.sync.dma_start(out=xt, in_=xpb[b, g * P:(g + 1) * P, :, :])
                        ssum = small.tile([P, 1], F32, tag="ssum")
                        # in-place identity copy on ACT; the free-dim sum falls out
                        # of accum_out, keeping the spatial sum off the DVE
                        nc.scalar.activation(out=xt, in_=xt, func=AF.Copy,
                                             accum_out=ssum)
                        nc.vector.tensor_scalar_mul(x2[:, 0, g, b:b + 1], ssum, 1.0 / (H * W))
                        nc.vector.tensor_reduce(
                            out=x2[:, 1, g, b:b + 1],
                            in_=xt[:, PAD:PAD + H, PAD:PAD + W], axis=AX.XY, op=OP.max)

                # ---- phase 2: stats -> kern ----
                xw = xw_g
                gx = gx_g
                for m in range(CG):  # output-channel group
                    ps = psum.tile([P, len(blist)], F32, tag="ps_stats")
                    k = 0
                    for s in range(2):
                        for g in range(CG):
                            nc.tensor.matmul(
                                ps, w2t_sb[:, s, g, m * P:(m + 1) * P], x2[:, s, g, blist[0]:blist[-1] + 1],
                                start=(k == 0), stop=(k == 5))
                            k += 1
                    # x_w = gelu(mm + b_avg + b_max)
                    nc.scalar.activation(out=xw[:, m, blist[0]:blist[-1] + 1], in_=ps, func=AF.Gelu,
                                         bias=bsum[:, m:m + 1], scale=1.0)
                    # Gx = |x_w| * wn  (wn > 0)
                    nc.scalar.activation(out=gx[:, m, blist[0]:blist[-1] + 1], in_=xw[:, m, blist[0]:blist[-1] + 1], func=AF.Abs,
                                         scale=wn[:, m:m + 1])

                # mean over channels via PE: ones^T @ gx sums partitions
                ps_sum = psum.tile([1, CG, len(blist)], F32, tag="ps_stats")
                nc.tensor.matmul(ps_sum, ones_col, gx[:, :, blist[0]:blist[-1] + 1],
                                 start=True, stop=True)
                gsum_sb = small.tile([1, CG, len(blist)], F32, tag="gsum")
                nc.vector.tensor_copy(out=gsum_sb.rearrange("p g b -> p (g b)"),
                                      in_=ps_sum.rearrange("p g b -> p (g b)"))
                rb1 = small.tile([1, len(blist)], F32, tag="rb1")
                nc.vector.tensor_reduce(out=rb1, in_=gsum_sb.rearrange("p g b -> p b g"),
                                        axis=AX.X, op=OP.add)
                # r = 1 / (mean + eps)
                nc.vector.tensor_scalar(out=rb1, in0=rb1, scalar1=1.0 / C,
                                        scalar2=EPS, op0=OP.mult, op1=OP.add)
                nc.vector.reciprocal(out=rb1, in_=rb1)
                # broadcast r to all partitions: ones_row^T @ rb1 -> [P, BL] in PSUM
                rb = psum.tile([P, len(blist)], F32, tag="ps_stats")
                nc.tensor.matmul(rb, ones_row, rb1, start=True, stop=True)

                # s = gamma * x_w * Gx * r ; kern = w_mix * s + beta
                kern = kern_g
                for m in range(CG):
                    t = small.tile([P, len(blist)], F32, tag="s_tmp")
                    nc.vector.tensor_mul(out=t, in0=xw[:, m, blist[0]:blist[-1] + 1], in1=gx[:, m, blist[0]:blist[-1] + 1])
                    nc.vector.tensor_mul(out=t, in0=t, in1=rb)
                    nc.vector.tensor_scalar_mul(t, t, gam_sb[:, m:m + 1])
                    for bi, b in enumerate(blist):
                        nc.vector.tensor_scalar(
                            out=kern[:, m, b, :], in0=wmx_sb[:, m, :],
                            scalar1=t[:, bi:bi + 1], scalar2=bet_sb[:, m:m + 1],
                            op0=OP.mult, op1=OP.add)


        # ---- phase 3: depthwise conv, 49 shifted MAC taps ----
        tiles = [(b, g) for b in range(BL) for g in range(CG)]

        def build_dms(b, g):
            dms = []
            for t in PE_TAPS:
                dm = dpool.tile([P, P], BF16, tag="diag")
                nc.vector.tensor_scalar(out=dm, in0=ident_bf,
                                        scalar1=kern[:, g, b, t:t + 1],
                                        scalar2=None, op0=OP.mult)
                dms.append(dm)
            return dms

        kern = kern_g
        phase12([0])
        dms_next = build_dms(*tiles[0])
        pending_res = []
        prev_accg = None
        emitted_second = False
        for ti_, (b, g) in enumerate(tiles):
            if ti_ == 1:
                phase12([1])
            elif ti_ == 4:
                phase12([2])
            elif ti_ == 7:
                phase12([3])
            xt = xres[:, b, g]
            dms = dms_next
            # DVE taps: fused STT into acc (bf16)
            acc = accp.tile([P, H, W], BF16, tag="acc")
            first_v = True
            for t in DVE_TAPS:
                di, dj = t // KW, t % KW
                xv = xt[:, di:di + H, dj:dj + W]
                ks = kern[:, g, b, t:t + 1]
                if first_v:
                    nc.vector.tensor_scalar(out=acc, in0=xv, scalar1=ks,
                                            scalar2=None, op0=OP.mult)
                    first_v = False
                else:
                    tv = tmpp.tile([P, H, W], BF16, tag="tmpv")
                    nc.vector.tensor_scalar(out=tv, in0=xv, scalar1=ks,
                                            scalar2=None, op0=OP.mult)
                    nc.vector.tensor_add(out=acc, in0=acc, in1=tv)
            # ACT taps -> bf16 tmp tiles consumed by PE ident-adds
            obs_k = small.tile([P, 1], F32, tag="obs_k")
            nc.scalar.copy(out=obs_k, in_=kern[:, g, b, 0:1])
            tmps = []
            for t in ACT_TAPS:
                di, dj = t // KW, t % KW
                tmp = tmpp.tile([P, H, W], BF16, tag="tmp16")
                nc.scalar.mul(out=tmp, in_=xt[:, di:di + H, dj:dj + W],
                              mul=kern[:, g, b, t:t + 1])
                tmps.append(tmp)
            # prebuild the NEXT tile's diag stationaries so the PE chain
            # never waits on DVE
            if ti_ + 1 < len(tiles):
                dms_next = build_dms(*tiles[ti_ + 1])
            # PE PSUM chain: per chunk-bank the sources are the 15 diag
            # taps, then the 14 ACT tmp adds, then identity-folds of the
            # DVE and Pool accumulators. Everything lands in PSUM, so the
            # store DMAs straight out of PSUM and no engine evicts.
            pacc = pep.tile([P, NCHUNK, 512], F32, tag="pe_acc")
            srcs = ([("diag", i) for i in range(len(PE_TAPS))] +
                    [("tmp", i) for i in range(len(ACT_TAPS))] +
                    [("fold", acc)])
            nsrc = len(srcs)
            for si, (kind, vv) in enumerate(srcs):
                if kind == "diag":
                    t = PE_TAPS[vv]
                    di, dj = t // KW, t % KW
                    stat = dms[vv]
                    def mov(c, di=di, dj=dj):
                        return xt[:, c * NROW + di: c * NROW + NROW + di,
                                  dj:dj + W]
                else:
                    src = tmps[vv] if kind == "tmp" else vv
                    stat = ident_bf
                    def mov(c, src=src):
                        return src[:, c * NROW:(c + 1) * NROW, :]
                for c in range(NCHUNK):
                    nc.tensor.matmul(
                        pacc[:, c, 0:NROW * W], stat, mov(c),
                        start=(si == 0), stop=(si == nsrc - 1))
            # ACT write-observer of the store from two tiles back: its
            # staging slot is the one this tile's evict will reuse, and the
            # anti-dep write threads the DMA completion into ACT's order
            if len(pending_res) >= 2:
                old = pending_res.pop(0)
                nc.scalar.mul(out=old[:, 0, 0:1], in_=old[:, 0, 0:1],
                              mul=1.0)
            # ACT evicts PSUM -> bf16 staging bank-by-bank as each bank's
            # accumulation group stops, so the next tile's chain restarts
            # almost immediately
            res = accp.tile([P, H, W], BF16, tag="res")
            resv = res.rearrange("p h w -> p (h w)").rearrange(
                "p (c x) -> p c x", x=NROW * W)
            for c in range(NCHUNK):
                nc.scalar.copy(out=resv[:, c, :], in_=pacc[:, c, 0:NROW * W])
            nc.sync.dma_start(out=outb[b, g * P:(g + 1) * P, :, :], in_=res)
            pending_res.append(res)
        # flush: observe the remaining stores on ACT so the final drain
        # carries a single wait
        for old in pending_res:
            nc.scalar.mul(out=old[:, 0, 0:1], in_=old[:, 0, 0:1], mul=1.0)

    import sys
    sys.setrecursionlimit(100000)
    _prune_redundant_dma_waits(nc)
    return nc


_NC_CACHE = {}


def _get_nc():
    if "nc" not in _NC_CACHE:
        _NC_CACHE["nc"] = _build_bass()
    return _NC_CACHE["nc"]


def _prep_inputs(x, w_avg, b_avg, w_max, b_max, w_mix, gamma, beta):
    x = np.asarray(x, dtype=np.float32)
    xp = np.zeros((B, C, HP, WP), dtype=np.float32)
    xp[:, :, PAD:PAD + H, PAD:PAD + W] = x
    xpb = xp.astype(ml_dtypes.bfloat16)
    w2t = np.ascontiguousarray(
        np.stack([np.asarray(w_avg, np.float32).T.reshape(CG, P, C),
                  np.asarray(w_max, np.float32).T.reshape(CG, P, C)]))
    b2 = np.ascontiguousarray(
        np.stack([np.asarray(b_avg, np.float32), np.asarray(b_max, np.float32)]))
    shared = {
        "w2t": w2t,
        "b2": b2,
        "wmx": np.ascontiguousarray(np.asarray(w_mix, np.float32)),
        "gam": np.ascontiguousarray(np.asarray(gamma, np.float32).reshape(C)),
        "bet": np.ascontiguousarray(np.asarray(beta, np.float32).reshape(C)),
        "ident": np.eye(P, dtype=ml_dtypes.bfloat16),
    }
    in_maps = []
    for i in range(NCORES):
        m = dict(shared)
        m["xpb"] = np.ascontiguousarray(xpb[i * BL:(i + 1) * BL])
        in_maps.append(m)
    return in_maps


def run(inputs, trace=False):
    nc = _get_nc()
    in_maps = _prep_inputs(**inputs)
    res = bass_utils.run_bass_kernel_spmd(
        nc, in_maps, core_ids=list(range(NCORES)), trace=trace)
    outs = [np.asarray(res.results[i]["outb"]).astype(np.float32)
            for i in range(NCORES)]
    full = np.concatenate(outs, axis=0)
    return full, res


def kernel(**inputs) -> np.ndarray:
    full, _ = run(inputs, trace=False)
    return full

